# revision 1
# baseline (speedup 1.0000x reference)
"""Trainium2 Bass kernel for the MINE-style segment_reduce problem.

Computes, for the fixed problem size B=16384, L=512, HID=768, TRANS=128:

    mask   = target.astype(f32)                     # [B, L] of {0,1}
    counts = max(mask.sum(1), 1)
    lf     = (mask @ label_embed) / counts          # [B, HID]
    net(t) = MLP(concat(t @ W_text.T + b_text, lf @ W_label.T + b_label))
    out    = mean(softplus(net(text[perm]))) + mean(softplus(-net(text)))

Algebraic folding (exact in real arithmetic): the first two linear layers
collapse into

    h1 = relu(text @ A_t.T + (mask @ LW2) / counts + c0)
    A_t = W0[:, :T] @ W_text                        # [T, HID]
    LW2 = (label_embed @ W_label.T) @ W0[:, T:].T   # [L, T]
    c0  = b0 + W0[:, :T] @ b_text + W0[:, T:] @ b_label

so label_embed never reaches the device; the per-sample network is two
small matmuls + relu + softplus.

Sharding: data-parallel over B across 8 NeuronCores (2048 rows each).
negative_text = text[perm] is realized host-side as a per-shard gather.
Each core returns the partial softplus sum over its rows; the host adds
8 scalars and divides by B.

Device-side design (v2):
 - All per-tile bulk data (mask | text | neg-text, fp8, pair-interleaved
   for DoubleRow) is packed host-side into ONE dram blob per batch tile,
   so the whole input streams in with 5 large HWDGE DMAs on the sync
   ring (HWDGE descriptor-gen is ~600ns of serial SP time per DMA - the
   v1 kernel spent ~17us there across 27 DMAs).
 - Weight-stationary matmul ordering: each DoubleRow fp8 weight pair is
   loaded once per 2-tile supertile and streams 4 matmuls (2 tiles x
   2 streams), instead of paying the 213ns LDWEIGHTS per matmul.
 - v = mask @ LW2 lands in the same PSUM bank that the joint-stream text
   matmuls later accumulate into (WAR handled by Tile), and the e rows
   land in the h2 banks, so the whole pipeline fits in 8 PSUM banks
   with 2-supertile double buffering.
 - 1/counts is applied as a bf16 row broadcast (one HWDGE DMA) and a DVE
   multiply; c0 and the relu fold into one DVE tensor_scalar; b1 folds
   into the h2 relu on ACT.
 - softplus runs directly on the [1,512] e-rows on ACT with accum_out,
   with +-b2 as the activation bias, so there is no staging copy, no
   repack DMA, and no final cross-partition matmul (v1 spent ~7us in
   that serialized tail).
"""

import numpy as np
import ml_dtypes

B, L, HID, TRANS = 16384, 512, 768, 128
NCORES = 8
BS = B // NCORES          # 2048 rows per core
BT = 512                  # batch tile (free-dim columns per PSUM bank)
NT = BS // BT             # 4 tiles per core
HC = HID // 128           # 6 contraction chunks for text
LC = L // 128             # 4 contraction chunks for the mask
HP = HC // 2              # 3 DoubleRow pairs for text
LP = LC // 2              # 2 DoubleRow pairs for the mask

MT_B = LP * 2 * BT        # 2048 mask bytes per partition per tile
XT_B = HP * 2 * BT        # 3072 text bytes per partition per tile
TILE_B = MT_B + 2 * XT_B  # 8192 = mask | neg-text | text

BF16 = ml_dtypes.bfloat16
FP8 = ml_dtypes.float8_e4m3

_CACHE = {}


def _split_sync_waits(nc, mybir, maxw_default=1, maxw_drain=1, maxw_types=None):
    """Walrus in this container rejects too many sync-waits per instruction
    ("Too many sync wait commands"); the limit varies by instruction type.
    Hoist excess waits onto NoOps that precede the instruction on the same
    engine."""
    maxw_types = maxw_types or {}
    for f in nc.m.functions:
        for bb in f.blocks:
            new = []
            for inst in bb.instructions:
                tn = type(inst).__name__
                if tn in ("InstDrain", "InstNoOp"):
                    maxw = maxw_drain
                else:
                    maxw = maxw_types.get(tn, maxw_default)
                si = inst.sync_info
                if si is not None and si.on_wait is not None and len(si.on_wait) > maxw:
                    waits = list(si.on_wait)
                    head, rest = waits[:-maxw], waits[-maxw:]
                    for k in range(0, len(head), maxw_drain):
                        nop = mybir.InstNoOp(name=f"{inst.name}-w{k}", ins=[], outs=[])
                        nop.engine = inst.engine
                        nop.sync_info = mybir.SyncInfo(
                            on_wait=head[k : k + maxw_drain], on_update=[]
                        )
                        new.append(nop)
                    inst.sync_info = mybir.SyncInfo(
                        on_wait=rest, on_update=list(si.on_update or [])
                    )
                new.append(inst)
            bb.instructions = new


N_WARM = 12


def _build():
    import concourse.bass as bass
    import concourse.mybir as mybir
    import concourse.tile as tile

    f32 = mybir.dt.float32
    bf16 = mybir.dt.bfloat16
    fp8 = mybir.dt.float8e4

    nc = bass.Bass("TRN2", target_bir_lowering=False, debug=False, num_devices=NCORES)

    WC8_B = (HC + LC) * TRANS    # 1280 bytes of fp8 weights at the blob front
    blob_d = nc.declare_dram_parameter("blob", [128, WC8_B + NT * TILE_B], fp8, isOutput=False)
    wc16_d = nc.declare_dram_parameter("wc16", [128, TRANS + 2], bf16, isOutput=False)
    cf_d = nc.declare_dram_parameter("cf", [TRANS, 5], f32, isOutput=False)
    cb_d = nc.declare_dram_parameter("cbv", [128, BS], bf16, isOutput=False)
    out_d = nc.declare_dram_parameter("out", [1, 1], f32, isOutput=True)

    AF = mybir.ActivationFunctionType
    ALU = mybir.AluOpType
    DR = mybir.MatmulPerfMode.DoubleRow

    with tile.TileContext(nc) as tc:
        with (
            tc.tile_pool(name="const", bufs=1) as cpool,
            tc.tile_pool(name="blob", bufs=1) as bpool,
            tc.tile_pool(name="vs", bufs=2) as vpool,
            tc.tile_pool(name="tmp", bufs=2) as tpool,
            tc.tile_pool(name="h1p", bufs=2) as h1pool,
            tc.tile_pool(name="h2p", bufs=2) as h2pool,
            tc.tile_pool(name="pu", bufs=2, space="PSUM") as pu,
            tc.tile_pool(name="pm", bufs=2, space="PSUM") as pm,
        ):
            # ---- constants, all on the scalar HWDGE ring (the gpsimd SWDGE
            # Q7 path adds ~4us of descriptor-emission lag). Order matters:
            # wc8 gates the first mask matmul; the 1/counts broadcast gates
            # the vs multiplies (and through the shared v/u PSUM bank, the
            # joint text matmuls). The broadcast is done SBUF->SBUF (load
            # the 4KB row first) - the HBM step-0 spray measured ~8us.
            # ---- all loads on ONE ring in strict priority order: HWDGE
            # descriptor generation is globally serial (~0.7us per DMA
            # instruction, regardless of which ring issues it), so a second
            # ring buys no parallelism - only ordering confusion. Only 7
            # DMAs issue before the tail (8 DMAHW semaphore lanes), so no
            # descriptor-gen ever waits on a completion. The tiny wc16/cf
            # consts go first (every elementwise op needs them); the fp8
            # weights ride the front of the blob (same gate as the first
            # mask matmul anyway). SBUF blob layout equals the DRAM layout
            # [wc8 A0 A1 B0 B1 A2 A3 B2 B3] (A = [mask|neg-text] per tile,
            # B = [text]), making every transfer a contiguous slice.
            A_B, B_B = MT_B + XT_B, XT_B
            ABASE = [WC8_B, WC8_B + A_B,
                     WC8_B + 2 * A_B + 2 * B_B, WC8_B + 3 * A_B + 2 * B_B]
            BBASE = [WC8_B + 2 * A_B, WC8_B + 2 * A_B + B_B,
                     WC8_B + 4 * A_B + 2 * B_B, WC8_B + 4 * A_B + 3 * B_B]
            big = bpool.tile([128, WC8_B + NT * TILE_B], fp8, tag="blob")

            wc16_sb = cpool.tile([128, TRANS + 2], bf16, tag="wc16")
            nc.sync.dma_start(wc16_sb[:], wc16_d[:, :])
            cf_sb = cpool.tile([TRANS, 5], f32, tag="cf")
            nc.sync.dma_start(cf_sb[:], cf_d[:, :])
            o = WC8_B + 2 * A_B
            nc.sync.dma_start(big[:, 0:o], blob_d[:, 0:o])   # wc8 + A0 + A1
            cb_sb = cpool.tile([128, BS], bf16, tag="cb")
            nc.sync.dma_start(cb_sb[:], cb_d[:, :])
            for ln in (2 * B_B, 2 * A_B, 2 * B_B):
                nc.sync.dma_start(big[:, o : o + ln], blob_d[:, o : o + ln])
                o += ln
            assert o == WC8_B + NT * TILE_B

            def w8p(c):    # fp8 weight chunk-pair [128, 2, TRANS] in the blob
                off = c * 2 * TRANS
                return big[:, off : off + 2 * TRANS].rearrange(
                    "p (c m) -> p c m", m=TRANS)

            def atTp(c):   # text pairs are chunks 0..5, mask pairs 6..9
                return w8p(c)

            def lw2p(c):
                return w8p(HC // 2 + c)

            w1T = wc16_sb[:, 0:TRANS]
            w2c = wc16_sb[:, TRANS : TRANS + 1]
            c0b = cf_sb[:, 0:1]
            b1b = cf_sb[:, 1:2]
            nb2 = cf_sb[0:1, 2:3]   # -b2 (row 0 scalar for the [1,*] e rows)
            pb2 = cf_sb[0:1, 3:4]   # +b2
            ones_col = cf_sb[:, 4:5]

            def pv(base):    # DoubleRow pair view [128, 2, BT] at byte offset
                return big[:, base : base + 2 * BT].rearrange("p (n j) -> p j n", j=2)

            def mt_ap(t, c):
                return pv(ABASE[t] + c * 2 * BT)

            def xn_ap(t, c):
                return pv(ABASE[t] + MT_B + c * 2 * BT)

            def xt_ap(t, c):
                return pv(BBASE[t] + c * 2 * BT)

            # ---- PE pre-warm: dummy matmuls with no input deps keep the PE
            # HAM activity window busy while the first loads are in flight.
            warm_sb = cpool.tile([128, BT], bf16, tag="warm")
            nc.vector.memset(warm_sb[:, :], 0)
            wp = pu.tile([128, 2 * BT], f32, tag="u")
            for _ in range(N_WARM):
                nc.tensor.matmul(wp[:, 0:BT], warm_sb[:, :TRANS], warm_sb[:, :],
                                 start=True, stop=True)

            NSEG = 2 * NT                     # 8 softplus row segments
            NPACK = 4                         # segments 0..3 go the packed path
            esp_sb = cpool.tile([1, NSEG * BT], f32, tag="esp")
            EPK = NPACK * BT // 128           # 16 packed cols
            epk_sb = cpool.tile([128, EPK], f32, tag="epk")
            acc2_sb = cpool.tile([128, 1], f32, tag="acc2")
            lnj_sb = cpool.tile([1, 2 * BT], f32, tag="lnj")
            accr_sb = cpool.tile([1, 3], f32, tag="accr")
            res_sb = cpool.tile([1, 1], f32, tag="res")

            # ---- main loop: supertiles of 2 batch tiles ----
            for S in range(NT // 2):
                ta, tb = 2 * S, 2 * S + 1
                u = {}
                vsb = {}
                for t in (ta, tb):
                    u[t] = pu.tile([128, 2 * BT], f32, tag="u", name=f"u{t}")
                # v = (mask @ LW2).T into the joint-u bank (freed by the vs
                # mul before the text matmuls overwrite it)
                for c in range(LP):
                    for t in (ta, tb):
                        nc.tensor.matmul(u[t][:, 0:BT], lw2p(c), mt_ap(t, c),
                                         start=(c == 0), stop=(c == LP - 1),
                                         perf_mode=DR)
                for t in (ta, tb):
                    vt = vpool.tile([128, 1, BT], f32, tag="vs")
                    nc.vector.tensor_mul(vt[:, 0, :], u[t][:, 0:BT],
                                         cb_sb[:, t * BT : (t + 1) * BT])
                    vsb[t] = vt
                # text matmuls, weight-stationary: one LDWEIGHTS per pair
                # streams 4 matmuls (2 tiles x {marginal, joint}). Marginal
                # first: the joint matmuls overwrite the v bank and so must
                # wait for the vs multiply (WAR).
                for c in range(HP):
                    for t in (ta, tb):
                        nc.tensor.matmul(u[t][:, BT:], atTp(c), xn_ap(t, c),
                                         start=(c == 0), stop=(c == HP - 1),
                                         perf_mode=DR)
                    for t in (ta, tb):
                        nc.tensor.matmul(u[t][:, 0:BT], atTp(c), xt_ap(t, c),
                                         start=(c == 0), stop=(c == HP - 1),
                                         perf_mode=DR)
                # elementwise + head, per tile. The first supertile uses
                # fused [128, 1024] ops (fewer instructions); the second
                # runs each stream's chain separately so the pipeline drain
                # after the last DMA is half as deep. exp folds in the +-b2
                # bias and the joint-stream negation (joint exp(-(e+b2)),
                # marginal exp(e+b2)), so the later ln(1+y) passes are
                # sign-agnostic and the packed layout may mix segments.
                for t in (ta, tb):
                    tmp = tpool.tile([128, 2 * BT], f32, tag="tmp")
                    h1 = h1pool.tile([128, 2 * BT], bf16, tag="h1")
                    hm = pm.tile([128, 2 * BT], f32, tag="hm")
                    h2s = h2pool.tile([128, 2 * BT], bf16, tag="h2s")
                    sj = 2 * t
                    if S == 0:
                        nc.vector.tensor_add(
                            tmp[:, :].rearrange("p (s n) -> p s n", s=2),
                            u[t][:, :].rearrange("p (s n) -> p s n", s=2),
                            vsb[t][:, :, :].broadcast_to([128, 2, BT]))
                        nc.vector.tensor_scalar(h1[:, :], tmp[:, :], c0b, 0.0,
                                                op0=ALU.add, op1=ALU.max)
                        nc.tensor.matmul(hm[:, 0:BT], w1T, h1[:, 0:BT], start=True, stop=True)
                        nc.tensor.matmul(hm[:, BT:], w1T, h1[:, BT:], start=True, stop=True)
                        nc.scalar.activation(h2s[:, :], hm[:, :], AF.Relu, bias=b1b)
                        # e rows land in row 0 of the (drained) h2 banks
                        nc.tensor.matmul(hm[0:1, 0:BT], w2c, h2s[:, 0:BT], start=True, stop=True)
                        nc.tensor.matmul(hm[0:1, BT:], w2c, h2s[:, BT:], start=True, stop=True)
                        nc.scalar.activation(esp_sb[:, sj * BT : (sj + 1) * BT],
                                             hm[0:1, 0:BT], AF.Exp,
                                             bias=nb2, scale=-1.0)
                        nc.scalar.activation(esp_sb[:, (sj + 1) * BT : (sj + 2) * BT],
                                             hm[0:1, BT:], AF.Exp, bias=pb2)
                        if sj + 2 == NPACK:
                            # supertile-0 segments complete: repack across
                            # partitions (this depends only on supertile-0
                            # work, so it issues before the S1 chains) and
                            # ln the [128, 16] block.
                            nc.sync.dma_start(epk_sb[:, :],
                                              esp_sb[:, 0 : NPACK * BT])
                            nc.scalar.activation(epk_sb[:, :], epk_sb[:, :],
                                                 AF.Ln, bias=1.0,
                                                 accum_out=acc2_sb[:, :])
                    else:
                        # marginal chain first (its u accumulation finishes
                        # before the joint one)
                        for s, (usl, bias, scale) in (
                                (1, (slice(BT, 2 * BT), pb2, 1.0)),
                                (0, (slice(0, BT), nb2, -1.0))):
                            nc.vector.tensor_add(tmp[:, usl], u[t][:, usl],
                                                 vsb[t][:, 0, :])
                            nc.vector.tensor_scalar(h1[:, usl], tmp[:, usl],
                                                    c0b, 0.0,
                                                    op0=ALU.add, op1=ALU.max)
                            nc.tensor.matmul(hm[:, usl], w1T, h1[:, usl],
                                             start=True, stop=True)
                            nc.scalar.activation(h2s[:, usl], hm[:, usl],
                                                 AF.Relu, bias=b1b)
                            nc.tensor.matmul(hm[0:1, usl], w2c, h2s[:, usl],
                                             start=True, stop=True)
                            nc.scalar.activation(
                                esp_sb[:, (sj + s) * BT : (sj + s + 1) * BT],
                                hm[0:1, usl], AF.Exp, bias=bias, scale=scale)
                            if t == 3:
                                # per-segment ln right behind each tail exp
                                # (marginal seg 7 first, joint seg 6 last)
                                nc.scalar.activation(
                                    lnj_sb[:, 0:BT],
                                    esp_sb[:, (sj + s) * BT : (sj + s + 1) * BT],
                                    AF.Ln, bias=1.0,
                                    accum_out=accr_sb[:, 1 + s : 2 + s])
                    if t == 2:
                        # segments 4..5 ln right after tile 2's exps
                        nc.scalar.activation(lnj_sb[:, :],
                                             esp_sb[:, 4 * BT : 6 * BT],
                                             AF.Ln, bias=1.0,
                                             accum_out=accr_sb[:, 0:1])

            res_ps = pm.tile([128, 2 * BT], f32, tag="hm")
            nc.tensor.matmul(res_ps[0:1, 0:1], acc2_sb[:, :], ones_col,
                             start=True, stop=True)
            nc.vector.tensor_add(res_sb[:, :], res_ps[0:1, 0:1], accr_sb[:, 0:1])
            nc.vector.tensor_add(res_sb[:, :], res_sb[:, :], accr_sb[:, 1:2])
            nc.vector.tensor_add(res_sb[:, :], res_sb[:, :], accr_sb[:, 2:3])
            nc.sync.dma_start(out_d[:, :], res_sb[:, :])

    # every instruction type in this walrus supports exactly ONE hw sync
    # wait (probed: DMACopy/Activation/Matmult/TensorTensor all reject 2+)
    _split_sync_waits(nc, mybir, maxw_default=1, maxw_drain=1)
    return nc


def _get_nc():
    if "nc" not in _CACHE:
        _CACHE["nc"] = _build()
    return _CACHE["nc"]


def _prep_inputs(text_embed, label_embed, target, perm,
                 W_text, b_text, W_label, b_label, W0, b0, W1, b1, W2, b2):
    f64 = np.float64
    W0t = W0[:, :TRANS].astype(f64)
    W0l = W0[:, TRANS:].astype(f64)
    A_t = W0t @ W_text.astype(f64)                                   # [T, HID]
    LW2 = (label_embed.astype(f64) @ W_label.T.astype(f64)) @ W0l.T  # [L, T]
    c0 = b0.astype(f64) + W0t @ b_text.astype(f64) + W0l @ b_label.astype(f64)

    atT_p = np.ascontiguousarray(A_t.T).reshape(HC, 128, TRANS).transpose(1, 0, 2).reshape(128, HID)
    lw2_p = np.ascontiguousarray(LW2).reshape(LC, 128, TRANS).transpose(1, 0, 2).reshape(128, L)
    wc8 = np.concatenate([atT_p, lw2_p], axis=1).astype(FP8)

    b2v = float(np.asarray(b2).reshape(-1)[0])
    wc16 = np.concatenate(
        [W1.T.astype(f64), W2.T.reshape(TRANS, 1).astype(f64), np.zeros((TRANS, 1))],
        axis=1).astype(BF16)                                         # [128, 130]
    cf = np.stack([c0, b1.astype(f64), np.full(TRANS, -b2v), np.full(TRANS, b2v),
                   np.ones(TRANS)], axis=1).astype(np.float32)       # [128, 5]

    counts = np.maximum(target.sum(axis=1), 1).astype(f64)
    cinv = (1.0 / counts).astype(BF16)                               # [B] bf16

    text_T = np.ascontiguousarray(text_embed.T).astype(FP8)          # [HID, B]
    mask_T = np.ascontiguousarray(target.T.astype(np.float32)).astype(FP8)  # [L, B]
    perm = np.asarray(perm).astype(np.int64)

    def interleave(a):
        # [2G*128, N] -> [128, G, 2N] fp8 with k-chunk pairs adjacent per column
        g2, n = a.shape[0] // 256, a.shape[1]
        return np.ascontiguousarray(
            a.reshape(g2, 2, 128, n).transpose(2, 0, 3, 1).reshape(128, g2, 2 * n)
        )

    in_maps = []
    for k in range(NCORES):
        sl = slice(k * BS, (k + 1) * BS)
        mtI = interleave(mask_T[:, sl])          # [128, LP, 2*BS]
        xtI = interleave(text_T[:, sl])          # [128, HP, 2*BS]
        xnI = interleave(text_T[:, perm[sl]])    # [128, HP, 2*BS]
        A, Bp = [], []
        for i in range(NT):
            sl2 = slice(2 * i * BT, 2 * (i + 1) * BT)
            A.append(np.concatenate(
                [mtI[:, :, sl2].reshape(128, -1),
                 xnI[:, :, sl2].reshape(128, -1)], axis=1))
            Bp.append(xtI[:, :, sl2].reshape(128, -1))
        # DRAM arrival order: wc8 A0 A1 B0 B1 A2 A3 B2 B3
        blob = np.ascontiguousarray(np.concatenate(
            [wc8, A[0], A[1], Bp[0], Bp[1], A[2], A[3], Bp[2], Bp[3]], axis=1))
        in_maps.append({
            "blob": blob,
            "wc16": wc16, "cf": cf,
            "cbv": np.ascontiguousarray(
                np.broadcast_to(cinv[sl].reshape(1, BS), (128, BS))),
        })
    return in_maps, b2v


def _run(in_maps, b2val, trace=False):
    from concourse.bass_utils import run_bass_kernel_spmd

    nc = _get_nc()
    res = run_bass_kernel_spmd(nc, in_maps, list(range(NCORES)), trace=trace)
    total = sum(float(res.results[k]["out"][0, 0]) for k in range(NCORES))
    return np.float32(total / B), res


def kernel(text_embed, label_embed, target, perm,
           W_text, b_text, W_label, b_label, W0, b0, W1, b1, W2, b2):
    in_maps, b2val = _prep_inputs(
        text_embed, label_embed, target, perm,
        W_text, b_text, W_label, b_label, W0, b0, W1, b1, W2, b2)
    out, _ = _run(in_maps, b2val)
    return out

